# revision 13
# baseline (speedup 1.0000x reference)
"""Trainium2 Bass kernel for nn_DriftingPolicy (Nadaraya-Watson RBF drift field).

Computes v = -drift(x, y_pos) + 0.5*drift(x, y_neg) where
  drift(x, y)_i = x_i * (s_i/denom_i) - (w @ y)_i / denom_i
  w_ij = exp(-||x_i - y_j||^2 / 2), diagonal (i==j) masked, s = rowsum(w),
  denom = max(s, 1e-8).

Sharding: rows of x (B=4096) split across 8 cores (512 rows each); y_pos/y_neg
replicated.  Per core, flash-style loop over 32 j-tiles of y:
  dist:  dot[j,i]  = sum_d y[j,d] x[i,d]        (PE, lhsT = y.T tile)
  w_raw = exp(dot - 0.5*||y_j||^2)              (ACT, per-partition bias)
  accT[d,i] += sum_j y[j,d] w_raw[j,i]          (PE, accumulating)
  s_raw[i]  += sum_j w_raw[j,i]                 (PE, ones-vector lhsT)
The per-i factor exp(-0.5*||x_i||^2) and the diagonal-mask correction
(subtract w_ii, computed directly from x and the core's diagonal y rows)
are folded into the epilogue scalars.  Host pre-transposes x and y so no
on-device data transposes are needed in the main loop.
"""

import numpy as np

B, TA, DA = 4096, 16, 8
D = TA * DA            # 128
NCORES = 8
IW = B // NCORES       # 512 query rows per core
P = 128                # partitions
NT = B // P            # 32 j-tiles
NCH = IW // P          # 4 i-chunks per core
EPS = 1e-8

_CACHE = {}


def _build(repeat=1):
    import concourse.bass as bass
    import concourse.tile as tile
    from concourse import mybir
    from concourse.masks import make_identity
    from concourse.bass import ts
    from contextlib import ExitStack

    F32 = mybir.dt.float32
    Alu = mybir.AluOpType
    Act = mybir.ActivationFunctionType

    nc = bass.Bass()
    x_d = nc.declare_dram_parameter("x", [IW, D], F32, isOutput=False)
    xT_d = nc.declare_dram_parameter("xT", [D, IW], F32, isOutput=False)
    y_d = [
        nc.declare_dram_parameter("y_pos", [B, D], F32, isOutput=False),
        nc.declare_dram_parameter("y_neg", [B, D], F32, isOutput=False),
    ]
    yT_d = [
        nc.declare_dram_parameter("yT_pos", [D, B], F32, isOutput=False),
        nc.declare_dram_parameter("yT_neg", [D, B], F32, isOutput=False),
    ]
    yd_d = [
        nc.declare_dram_parameter("yd_pos", [IW, D], F32, isOutput=False),
        nc.declare_dram_parameter("yd_neg", [IW, D], F32, isOutput=False),
    ]
    ysq_d = [
        nc.declare_dram_parameter("ysqh_pos", [P, NT], F32, isOutput=False),
        nc.declare_dram_parameter("ysqh_neg", [P, NT], F32, isOutput=False),
    ]
    out_d = nc.declare_dram_parameter("out", [IW, D], F32, isOutput=True)

    with tile.TileContext(nc) as tc, ExitStack() as ctx:
        singles = ctx.enter_context(tc.tile_pool(name="singles", bufs=1))
        wpool = ctx.enter_context(tc.tile_pool(name="wpool", bufs=4))
        scrpool = ctx.enter_context(tc.tile_pool(name="scr", bufs=2))
        ps_dot = ctx.enter_context(tc.tile_pool(name="ps_dot", bufs=3, space="PSUM"))
        ps_acc = ctx.enter_context(tc.tile_pool(name="ps_acc", bufs=2, space="PSUM"))
        ps_s = ctx.enter_context(tc.tile_pool(name="ps_s", bufs=2, space="PSUM"))
        epi = ctx.enter_context(tc.tile_pool(name="epi", bufs=2))

        # ---- constants & inputs resident in SBUF ----
        ident = singles.tile([P, P], F32, name="ident", tag="ident")
        make_identity(nc, ident[:, :])
        ones = singles.tile([P, 1], F32, name="ones", tag="ones")
        nc.gpsimd.memset(ones[:, :], 1.0)

        # x in chunk-major layout [p, ch, d]  (row i = ch*128 + p)
        x_sb = singles.tile([P, NCH, D], F32, name="x_sb", tag="x_sb")
        nc.sync.dma_start(x_sb[:, :, :], x_d[:, :].rearrange("(c p) d -> p c d", p=P))
        xT_sb = singles.tile([D, IW], F32, name="xT_sb", tag="xT_sb")
        nc.sync.dma_start(xT_sb[:, :], xT_d[:, :])

        yd_sb = []
        for f in range(2):
            t = singles.tile([P, NCH, D], F32, name=f"yd{f}", tag=f"yd{f}")
            nc.sync.dma_start(t[:, :, :], yd_d[f][:, :].rearrange("(c p) d -> p c d", p=P))
            yd_sb.append(t)

        y_sb = []
        yT_sb = []
        for f in range(2):
            ty = singles.tile([P, NT, D], F32, name=f"y{f}", tag=f"y{f}")
            y_ap = y_d[f][:, :].rearrange("(t p) d -> p t d", p=P)
            HEAD = 8
            nc.sync.dma_start(ty[:, 0:HEAD, :], y_ap[:, 0:HEAD, :])
            nc.sync.dma_start(ty[:, HEAD:NT, :], y_ap[:, HEAD:NT, :])
            y_sb.append(ty)
            tyT = singles.tile([D, B], F32, name=f"yT{f}", tag=f"yT{f}")
            nc.sync.dma_start(tyT[:, 0 : HEAD * P], yT_d[f][:, 0 : HEAD * P])
            nc.sync.dma_start(tyT[:, HEAD * P : B], yT_d[f][:, HEAD * P : B])
            yT_sb.append(tyT)
        ysq_sb = []
        for f in range(2):
            tq = singles.tile([P, NT], F32, name=f"ysq{f}", tag=f"ysq{f}")
            nc.sync.dma_start(tq[:, :], ysq_d[f][:, :])
            ysq_sb.append(tq)

        # ---- per-row scalars: xsqh = -0.5*||x_i||^2, exb = exp(xsqh),
        #      wii_f = exp(-0.5*||x_i - ydiag_i||^2) ----
        xsq = singles.tile([P, NCH], F32, name="xsq", tag="xsq")
        for ch in range(NCH):
            scr = scrpool.tile([P, D], F32, name="scr", tag="scr")
            nc.vector.tensor_mul(scr[:, :], x_sb[:, ch, :], x_sb[:, ch, :])
            nc.vector.reduce_sum(
                xsq[:, ch : ch + 1], scr[:, :], axis=mybir.AxisListType.X
            )
        exb = singles.tile([P, NCH], F32, name="exb", tag="exb")
        nc.scalar.activation(exb[:, :], xsq[:, :], Act.Exp, scale=-0.5)

        wii = []
        for f in range(2):
            d2 = singles.tile([P, NCH], F32, name=f"d2_{f}", tag=f"d2_{f}")
            for ch in range(NCH):
                diff = scrpool.tile([P, D], F32, name="diff", tag="scr")
                nc.vector.tensor_sub(diff[:, :], x_sb[:, ch, :], yd_sb[f][:, ch, :])
                scr2 = scrpool.tile([P, D], F32, name="scr2", tag="scr")
                nc.vector.tensor_mul(scr2[:, :], diff[:, :], diff[:, :])
                nc.vector.reduce_sum(
                    d2[:, ch : ch + 1], scr2[:, :], axis=mybir.AxisListType.X
                )
            w = singles.tile([P, NCH], F32, name=f"wii{f}", tag=f"wii{f}")
            nc.scalar.activation(w[:, :], d2[:, :], Act.Exp, scale=-0.5)
            wii.append(w)

        # ---- main loop: two fields, 32 j-tiles each ----
        accT_sb = []   # [d, i] accumulators copied to SBUF
        srows = [
            singles.tile([1, IW], F32, name="srow0", tag="srow0"),
            singles.tile([1, IW], F32, name="srow1", tag="srow1"),
        ]
        def emit_dist(f, t):
            dot_ps = ps_dot.tile([P, IW], F32, name="dot_ps", tag="dot")
            nc.tensor.matmul(
                dot_ps[:, :], lhsT=yT_sb[f][:, ts(t, P)], rhs=xT_sb[:, :],
                start=True, stop=True,
            )
            return dot_ps

        def emit_exp(f, t, dot_ps):
            w_t = wpool.tile([P, IW], F32, name="w_t", tag="w")
            nc.scalar.activation(
                w_t[:, :], dot_ps[:, :], Act.Exp,
                bias=ysq_sb[f][:, t : t + 1], scale=1.0,
            )
            return w_t

        # software pipeline across both fields: dist runs DEPTH iterations
        # ahead of acc/s, exp runs in between, so PE and ACT never ping-pong.
        steps = [(f, t) for f in range(2) for t in range(NT)] * repeat
        DEPTH = 2
        dots = {}
        ws = {}
        accT_ps_f = {}
        s_ps_f = {}
        for f in range(2):
            accT_ps_f[f] = ps_acc.tile([P, IW], F32, name="accT_ps", tag="acc")
            s_ps_f[f] = ps_s.tile([1, IW], F32, name="s_ps", tag="s")
        for k in range(DEPTH):
            dots[k] = emit_dist(*steps[k])
            ws[k] = emit_exp(*steps[k], dots[k])
        for i, (f, t) in enumerate(steps):
            if i + DEPTH < len(steps):
                dots[i + DEPTH] = emit_dist(*steps[i + DEPTH])
                ws[i + DEPTH] = emit_exp(*steps[i + DEPTH], dots[i + DEPTH])
            w_t = ws.pop(i)
            dots.pop(i)
            nc.tensor.matmul(
                accT_ps_f[f][:, :], lhsT=y_sb[f][:, t, :], rhs=w_t[:, :],
                start=(t == 0), stop=(t == NT - 1),
            )
            nc.tensor.matmul(
                s_ps_f[f][:, :], lhsT=ones[:, :], rhs=w_t[:, :],
                start=(t == 0), stop=(t == NT - 1),
            )

        accT_ps_l = [accT_ps_f[0], accT_ps_f[1]]
        s_ps_l = [s_ps_f[0], s_ps_f[1]]
        for f in range(2):
            accT_ps = accT_ps_l[f]
            s_ps = s_ps_l[f]
            acc_sb = epi.tile([P, IW], F32, name="acc_sb", tag="accsb", bufs=2)
            nc.scalar.copy(acc_sb[:, :], accT_ps[:, :])
            accT_sb.append(acc_sb)
            nc.scalar.copy(srows[f][:, :], s_ps[:, :])

        # ---- epilogue ----
        # transpose s rows -> per-partition scalars sT[p, ch, f]
        sT_ps = ps_s.tile([P, NCH, 2], F32, name="sT_ps", tag="s")
        for k in range(2 * NCH):
            ch, f = divmod(k, 2)
            nc.tensor.matmul(
                sT_ps[:, ch, f : f + 1], lhsT=srows[f][0:1, ts(ch, P)],
                rhs=ident[0:1, 0:1],
                is_transpose=True, start=(k == 0), stop=(k == 2 * NCH - 1),
            )
        sT_sb = singles.tile([P, NCH, 2], F32, name="sT_sb", tag="sT_sb")
        nc.vector.tensor_copy(sT_sb[:, :, :], sT_ps[:, :, :])

        # transpose accT [d, i] -> acc [i, d] per chunk (into [p, ch, d] layout)
        accTr_ps = []
        for f in range(2):
            tr = ps_dot.tile([P, NCH, P], F32, name="tr", tag="dot")
            for ch in range(NCH):
                nc.tensor.matmul(
                    tr[:, ch, :], lhsT=accT_sb[f][:, ts(ch, P)], rhs=ident[:, :],
                    is_transpose=True, start=(ch == 0), stop=(ch == NCH - 1),
                )
            accTr_ps.append(tr)

        # scalar math on [P, NCH] tiles
        def small(tag):
            return singles.tile([P, NCH], F32, name=tag, tag=tag)

        rr = []          # r_f = 1/denom_f
        ratio = []       # ratio_f = s_f/denom_f
        for f in range(2):
            sraw = sT_sb[:, :, f]
            st = small(f"st{f}")
            nc.vector.tensor_mul(st[:, :], sraw, exb[:, :])          # exb*s_raw
            nc.vector.tensor_sub(st[:, :], st[:, :], wii[f][:, :])   # - w_ii
            dn = small(f"dn{f}")
            nc.vector.tensor_scalar_max(dn[:, :], st[:, :], EPS)
            r = small(f"r{f}")
            nc.vector.reciprocal(r[:, :], dn[:, :])
            ra = small(f"ra{f}")
            nc.vector.tensor_mul(ra[:, :], st[:, :], r[:, :])
            rr.append(r)
            ratio.append(ra)

        coefx = small("coefx")     # 0.5*ratio_n - ratio_p
        nc.vector.scalar_tensor_tensor(
            out=coefx[:, :], in0=ratio[1][:, :], scalar=0.5, in1=ratio[0][:, :],
            op0=Alu.mult, op1=Alu.subtract,
        )
        apscale = small("apscale")  # exb * r_p
        nc.vector.tensor_mul(apscale[:, :], exb[:, :], rr[0][:, :])
        anscale = small("anscale")  # -0.5 * exb * r_n
        nc.vector.scalar_tensor_tensor(
            out=anscale[:, :], in0=rr[1][:, :], scalar=-0.5, in1=exb[:, :],
            op0=Alu.mult, op1=Alu.mult,
        )
        pdscale = small("pdscale")  # -wii_p * r_p
        nc.vector.scalar_tensor_tensor(
            out=pdscale[:, :], in0=wii[0][:, :], scalar=-1.0, in1=rr[0][:, :],
            op0=Alu.mult, op1=Alu.mult,
        )
        ndscale = small("ndscale")  # +0.5 * wii_n * r_n
        nc.vector.scalar_tensor_tensor(
            out=ndscale[:, :], in0=wii[1][:, :], scalar=0.5, in1=rr[1][:, :],
            op0=Alu.mult, op1=Alu.mult,
        )

        # final combine per chunk:
        # v = x*coefx + accTr_p*apscale + accTr_n*anscale + ypd*pdscale + ynd*ndscale
        out_sb = singles.tile([P, NCH, D], F32, name="out_sb", tag="out_sb")
        for ch in range(NCH):
            ta = epi.tile([P, D], F32, name="ta", tag="ta")
            tb = epi.tile([P, D], F32, name="tb", tag="tb")
            nc.vector.tensor_scalar_mul(ta[:, :], x_sb[:, ch, :], coefx[:, ch : ch + 1])
            nc.vector.scalar_tensor_tensor(
                out=tb[:, :], in0=accTr_ps[0][:, ch, :], scalar=apscale[:, ch : ch + 1],
                in1=ta[:, :], op0=Alu.mult, op1=Alu.add,
            )
            ta2 = epi.tile([P, D], F32, name="ta2", tag="ta")
            nc.vector.scalar_tensor_tensor(
                out=ta2[:, :], in0=accTr_ps[1][:, ch, :], scalar=anscale[:, ch : ch + 1],
                in1=tb[:, :], op0=Alu.mult, op1=Alu.add,
            )
            tb2 = epi.tile([P, D], F32, name="tb2", tag="tb")
            nc.vector.scalar_tensor_tensor(
                out=tb2[:, :], in0=yd_sb[0][:, ch, :], scalar=pdscale[:, ch : ch + 1],
                in1=ta2[:, :], op0=Alu.mult, op1=Alu.add,
            )
            nc.vector.scalar_tensor_tensor(
                out=out_sb[:, ch, :], in0=yd_sb[1][:, ch, :], scalar=ndscale[:, ch : ch + 1],
                in1=tb2[:, :], op0=Alu.mult, op1=Alu.add,
            )

        nc.sync.dma_start(out_d[:, :].rearrange("(c p) d -> p c d", p=P), out_sb[:, :, :])

    return nc


def _split_multi_waits(nc):
    """The walrus build behind the PJRT path accepts at most ONE sync-wait per
    instruction (setupSyncWait 'Too many sync wait commands').  Hoist extra
    waits onto preceding same-engine NoOps, which each carry one wait."""
    from concourse import mybir

    for bb in nc.m.functions[0].blocks:
        out = []
        for inst in bb.instructions:
            si = inst.sync_info
            if (
                si is not None and si.on_wait and len(si.on_wait) > 1
                and type(inst).__name__ != "InstNoOp"
            ):
                waits = list(si.on_wait)
                for k, w in enumerate(waits[:-1]):
                    out.append(mybir.InstNoOp(
                        name=f"{inst.name}-wsplit{k}",
                        engine=inst.engine,
                        ins=[], outs=[],
                        sync_info=mybir.SyncInfo(on_wait=[w], on_update=[]),
                    ))
                si.on_wait = waits[-1:]
            out.append(inst)
        bb.instructions[:] = out
    return nc


def _get_nc(repeat=1):
    key = f"nc{repeat}"
    if key not in _CACHE:
        _CACHE[key] = _split_multi_waits(_build(repeat))
    return _CACHE[key]


def _get_raw_nc():
    """Unsplit build for CoreSim (which rejects wait-only NoOps)."""
    if "nc_raw" not in _CACHE:
        _CACHE["nc_raw"] = _build()
    return _CACHE["nc_raw"]


def _in_maps(x, y_pos, y_neg):
    xf = np.ascontiguousarray(np.asarray(x, dtype=np.float32).reshape(B, D))
    ypf = np.ascontiguousarray(np.asarray(y_pos, dtype=np.float32).reshape(B, D))
    ynf = np.ascontiguousarray(np.asarray(y_neg, dtype=np.float32).reshape(B, D))
    ypT = np.ascontiguousarray(ypf.T)
    ynT = np.ascontiguousarray(ynf.T)

    def _ysqh(yf):
        h = (-0.5 * (yf.astype(np.float64) ** 2).sum(axis=1)).astype(np.float32)
        return np.ascontiguousarray(h.reshape(NT, P).T)

    ysqh_p = _ysqh(ypf)
    ysqh_n = _ysqh(ynf)
    maps = []
    for c in range(NCORES):
        sl = slice(c * IW, (c + 1) * IW)
        maps.append({
            "x": xf[sl],
            "xT": np.ascontiguousarray(xf[sl].T),
            "y_pos": ypf,
            "y_neg": ynf,
            "yT_pos": ypT,
            "yT_neg": ynT,
            "yd_pos": ypf[sl],
            "yd_neg": ynf[sl],
            "ysqh_pos": ysqh_p,
            "ysqh_neg": ysqh_n,
        })
    return maps


def _run(in_maps, trace=False, **kw):
    from concourse.bass_utils import run_bass_kernel_spmd

    nc = _get_nc()
    return run_bass_kernel_spmd(nc, in_maps, list(range(NCORES)), trace=trace, **kw)


def kernel(x, y_pos, y_neg):
    res = _run(_in_maps(x, y_pos, y_neg))
    out = np.concatenate([res.results[c]["out"] for c in range(NCORES)], axis=0)
    return out.reshape(B, TA, DA).astype(np.float32)
